# revision 30
# baseline (speedup 1.0000x reference)
"""HOSVD aggregator kernel for 8 TRN2 NeuronCores.

y[n,o] = sum_{m0..m4} G[m0,m1,m2,m3,m4] * ris0[n,m0] * ris1[n,m4]
         * ris2[n,m3] * ris3[n,m2] * U_out[m1,o],
with ris_d = X[:,d,:] @ U_stack[d].

Strategy: data-parallel over nodes (6250/core), transposed layout
(features/tensor-rows on SBUF partitions, nodes on the free dim).
With z01[a=(m0 m4)] = ris0[m0]*ris1[m4], z23[b=(m3 m2)] = ris2[m3]*ris3[m2]
and Ghat[m1,a,b] = G[m0,m1,m2,m3,m4]:
  s[n,m1] = z01^T Ghat[m1] z23;  y = U_out^T s.
Mixed-chunk formulation: for m1 in {4..7} contract z23 first (rows = a,
then multiply z01); for m1 in {0..3} contract z01 first (rows = b, then
multiply z23). Even chunks use a [B-side | A-side] psum layout, odd
chunks the flipped [A | B] layout, so a chunk PAIR's four K=64/M=64 PT
matmuls occupy the four disjoint PE quadrants and stream concurrently;
the elementwise multiply uses zzp = [z01|z23 | swapped] (swap halves
built by one DVE 4x copy + one ScalarE copy). Factor matmuls are M=64
col-tile pairs at (0,0)/(0,64) into one 2-bank psum tile, staged to
SBUF bf16 by a single ScalarE copy so the z-product runs in DVE 2x mode.
Pipeline: 2-deep software pipeline (PE queue per iteration is
factors(s+1) | PT(s) | y(s-1)) so every matmul's cross-engine inputs are
a full stage old. DMA: all input supertiles queued upfront on the sync
HWDGE ring (FIFO, tail first); outputs (bf16, 2-supertile batches, all
staging buffers live) queue on the same ring BEHIND the inputs so they
never steal HBM bandwidth from the input stream. PE warm-up matmuls on
a memset tile right after the preamble defeat the HAM cold-clock window.
"""

import sys

sys.path.insert(0, "/opt/trn_rl_repo")

import os
import numpy as np
import ml_dtypes

import concourse.bass as bass
import concourse.tile as tile
from concourse import mybir
from concourse.bass_utils import run_bass_kernel_spmd

BF16 = ml_dtypes.bfloat16

N = 50000
NCORES = 8
NPC = N // NCORES            # 6250 nodes per core
T = 512                      # nodes per supertile
NSUP = NPC // T              # 12 full supertiles
TAIL = NPC - NSUP * T        # 106
BATCH = 2                    # supertiles per output DMA
NB = NSUP // BATCH           # 3 output batches
NWARM = 20                   # PE warm-up matmuls (HAM un-throttle)

# ---------------------------------------------------------------------------
# walrus rejects >1 sync wait on a Drain; Tile's tail drain carries one wait
# per logical proc. Split it into a chain of single-wait drains.
import bass_rust as _br
from concourse.vector_clock import ScopedClock as _ScopedClock


def _split_drain_and_barrier(self, tick_clock, wait_clock):
    drain_inst = self.nc.sync.drain()
    wait_clock.add_sem_waits(
        drain_inst.ins, _ScopedClock({None: tick_clock.global_clock})
    )
    si = drain_inst.ins.sync_info
    waits = list(si.on_wait)
    if len(waits) > 1:
        drain_inst.ins.sync_info = _br.SyncInfo(on_wait=waits[:1], on_update=[])
        rest = waits[1:]
        while rest:
            d2 = self.nc.sync.drain()
            chunk, rest = rest[:1], rest[1:]
            d2.ins.sync_info = _br.SyncInfo(
                on_wait=chunk, on_update=list(si.on_update) if not rest else []
            )
    self.nc.all_engine_barrier()
    assert self.sems is not None
    popped = self.nc._tile_sem_poison_stack.pop()
    assert popped is self._sem_poison
    self.nc.clear_and_free_semaphores(list(self.sems.allocated().values()))
    self.nc.all_engine_barrier()


tile.TileContext._drain_and_barrier = _split_drain_and_barrier

# Same walrus limit applies to every instruction type: peel extra sem waits
# onto single-wait NOPs emitted just before the instruction, same engine.
_SPLIT_SEQ = [0]
_orig_add_instruction = tile.TileContext._add_instruction


def _split_add_instruction(self, inst):
    si = inst.sync_info
    waits = list(si.on_wait) if si is not None else []
    if len(waits) > 1:
        for w in waits[:-1]:
            _SPLIT_SEQ[0] += 1
            nop = mybir.InstNoOp(name=f"waitsplit_{_SPLIT_SEQ[0]}", ins=[],
                                 outs=[], engine=inst.engine)
            nop.sync_info = _br.SyncInfo(on_wait=[w], on_update=[])
            _orig_add_instruction(self, nop)
        inst.sync_info = _br.SyncInfo(on_wait=[waits[-1]],
                                      on_update=list(si.on_update))
    return _orig_add_instruction(self, inst)


tile.TileContext._add_instruction = _split_add_instruction

# ---------------------------------------------------------------------------
# weight-pack free-dim offsets (all bf16, one [128, 1280] SBUF tile)
_WA = 0      # 2 chunks [128,128]: cols 0-63 A01 (W0), 64-127 A23 (W2)
_WB = 256    # 2 chunks [128,128]: cols 0-63 B01 (W1), 64-127 B23 (W3)
_WG = 512    # 4 blocks [128,64]: rows 0-63 GA_c=Ghat[c], rows 64-127 GB_c=Ghat[4+c].T
_WU = 768    # 4 chunks [128,128]: rows 0-63 U_out[4+c], rows 64-127 U_out[c]
_WCOLS = 1280


def _build_nc():
    nc = bass.Bass("TRN2", target_bir_lowering=False, debug=False,
                   num_devices=NCORES)
    bf = mybir.dt.bfloat16
    f32 = mybir.dt.float32

    xm = nc.dram_tensor("xm", [NSUP, 128, 8 * T], bf, kind="ExternalInput").ap()
    xt = nc.dram_tensor("xt", [128, 8 * TAIL], bf, kind="ExternalInput").ap()
    wp = nc.dram_tensor("wp", [128, _WCOLS], bf, kind="ExternalInput").ap()
    ym = nc.dram_tensor("ym", [NB, 128, BATCH * T], bf, kind="ExternalOutput").ap()
    yt = nc.dram_tensor("yt", [128, TAIL], bf, kind="ExternalOutput").ap()

    with tile.TileContext(nc) as tc:
        from contextlib import ExitStack
        with ExitStack() as ctx:
            wpool = ctx.enter_context(tc.tile_pool(name="w", bufs=1))
            xpool = ctx.enter_context(tc.tile_pool(name="x", bufs=1))
            spool = ctx.enter_context(tc.tile_pool(name="s", bufs=3))
            qpool = ctx.enter_context(tc.tile_pool(name="q", bufs=3))
            ybpool = ctx.enter_context(tc.tile_pool(name="yb", bufs=NB))
            pfAB = ctx.enter_context(tc.tile_pool(name="pfAB", bufs=1,
                                                   space="PSUM"))
            ppt = ctx.enter_context(tc.tile_pool(name="ppt", bufs=2, space="PSUM"))
            pyp = ctx.enter_context(tc.tile_pool(name="py", bufs=2, space="PSUM"))

            ws = wpool.tile([128, _WCOLS], bf)
            nc.sync.dma_start(ws[:], wp[:])

            # queue every input DMA up front on the sync HWDGE ring (FIFO);
            # transfers then stream back-to-back at full HBM rate. The small
            # tail goes first so its compute fills the pipeline ramp instead
            # of dangling off the end.
            order = [NSUP] + list(range(NSUP))
            xs_tiles = {}
            for s in order:
                tc_ = T if s < NSUP else TAIL
                xs = xpool.tile([128, 8 * tc_], bf, tag=f"xs{s}")
                nc.sync.dma_start(xs[:], xm[s] if s < NSUP else xt[:])
                xs_tiles[s] = xs

            # PE warm-up: dummy matmuls on a memset scratch tile (no DMA
            # dependency, so they start right after the preamble); ~3.4us of
            # PE-busy flips HAM to full clock, and the run length is sized so
            # the PE stays warm until the first supertile's matmuls issue.
            # A dummy GpSimd tensor op in the same window absorbs the
            # one-time Q7 ucode load off the critical path.
            wt = wpool.tile([128, T], bf, tag="warm")
            nc.gpsimd.memset(wt[:], 0)
            wg = wpool.tile([128, 16], bf, tag="warmg")
            nc.gpsimd.tensor_mul(wg[:], wt[:, 0:16], wt[:, 16:32])
            for w in range(NWARM):
                pw = pyp.tile([128, T], f32, tag="psy")
                nc.tensor.matmul(pw[:], wt[:, 0:128], wt[:, 0:T],
                                 start=True, stop=True)

            # ---- software-pipelined main loop -------------------------------
            # emit_front(s): factor matmuls + ScalarE psum->sbuf bf16 staging
            #   + DVE 2x-mode zz mul (all-bf16 SBUF operands).
            # emit_back(s):  PT pairs, merged q muls, y accumulation, output.
            # Front of supertile i+1 is emitted BEFORE back of supertile i so
            # the DVE/PE/ACT queues always hold ready cross-supertile work.
            yb_holder = [None]

            def emit_front(s):
                tc_ = T if s < NSUP else TAIL
                xs = xs_tiles[s]

                def xc(d, k):
                    return xs[:, (2 * d + k) * tc_:(2 * d + k + 1) * tc_]

                # (PSUM tiles are full-bank-width [*, T], sliced for the
                # tail: a matmul's start=True zeroes a whole 2KB bank region,
                # so sub-bank-packed tiles would clobber each other)
                # emission order interleaves the (0,0)/(0,64) col-tile pair
                # per K-chunk so adjacent PE-queue entries run concurrently
                psABf = pfAB.tile([128, 2 * T], f32, tag="psAB")
                psA = psABf[:, 0:tc_]
                nc.tensor.matmul(psA[0:64, :], ws[:, _WA:_WA + 64],
                                 xc(0, 0), start=True, stop=False)
                nc.tensor.matmul(psA[0:64, :], ws[:, _WA + 128:_WA + 192],
                                 xc(0, 1), start=False, stop=True)
                nc.tensor.matmul(psA[64:128, :], ws[:, _WA + 64:_WA + 128],
                                 xc(2, 0), start=True, stop=False,
                                 tile_position=(0, 64))
                nc.tensor.matmul(psA[64:128, :], ws[:, _WA + 192:_WA + 256],
                                 xc(2, 1), start=False, stop=True,
                                 tile_position=(0, 64))
                psB = psABf[:, T:T + tc_]
                nc.tensor.matmul(psB[0:64, :], ws[:, _WB:_WB + 64],
                                 xc(1, 0), start=True, stop=False)
                nc.tensor.matmul(psB[0:64, :], ws[:, _WB + 128:_WB + 192],
                                 xc(1, 1), start=False, stop=True)
                nc.tensor.matmul(psB[64:128, :], ws[:, _WB + 64:_WB + 128],
                                 xc(3, 0), start=True, stop=False,
                                 tile_position=(0, 64))
                nc.tensor.matmul(psB[64:128, :], ws[:, _WB + 192:_WB + 256],
                                 xc(3, 1), start=False, stop=True,
                                 tile_position=(0, 64))

                # stage both factor tiles to SBUF as bf16 (frees the PSUM
                # banks immediately; all-16-bit SBUF operands let the zz mul
                # run in DVE 2x_1P mode). zzp = [zz | zzswap] where zzswap
                # has the partition halves exchanged (GpSimd cross-partition
                # copies) - the swap feeds the odd (flipped-layout) chunks'
                # elementwise multiply.
                sAB = spool.tile([128, 2 * tc_], bf, tag="sAB")
                nc.scalar.copy(
                    sAB[:].rearrange("p (c t) -> p c t", c=2),
                    psABf[:].rearrange("p (c t) -> p c t", c=2)[:, :, 0:tc_])
                zzp = spool.tile([128, 2 * tc_], bf, tag="zz")
                nc.vector.tensor_mul(zzp[:, 0:tc_], sAB[:, tc_:2 * tc_],
                                     sAB[:, 0:tc_])
                nc.scalar.copy(zzp[0:64, tc_:2 * tc_],
                               zzp[64:128, 0:tc_])

                # the ScalarE swap-half is emitted AFTER emit_y so the ys
                # output copy is not queued behind its long DVE wait
                def act_swap():
                    nc.scalar.copy(zzp[64:128, tc_:2 * tc_],
                                   zzp[0:64, 0:tc_])
                return zzp, act_swap

            def emit_pt(s, zzp):
                tc_ = T if s < NSUP else TAIL
                z01 = zzp[0:64, 0:tc_]
                z23 = zzp[64:128, 0:tc_]
                qs = []
                for h in range(2):
                    # chunk pair (2h, 2h+1): even chunk = [B | A] layout at
                    # quadrants (64,0)/(0,64); odd chunk = flipped [A | B] at
                    # (0,0)/(64,64). All four matmuls occupy disjoint PE
                    # quadrants and stream concurrently.
                    pt = ppt.tile([128, 2 * T], f32, tag="pt")
                    ce, co = 2 * h, 2 * h + 1
                    sl0 = pt[:, 0:tc_]
                    sl1 = pt[:, T:T + tc_]
                    nc.tensor.matmul(sl0[0:64, :],
                                     ws[64:128, _WG + 64 * ce:_WG + 64 * (ce + 1)],
                                     z23, start=True, stop=True,
                                     tile_position=(64, 0))
                    nc.tensor.matmul(sl0[64:128, :],
                                     ws[0:64, _WG + 64 * ce:_WG + 64 * (ce + 1)],
                                     z01, start=True, stop=True,
                                     tile_position=(0, 64))
                    nc.tensor.matmul(sl1[0:64, :],
                                     ws[0:64, _WG + 64 * co:_WG + 64 * (co + 1)],
                                     z01, start=True, stop=True,
                                     tile_position=(0, 0))
                    nc.tensor.matmul(sl1[64:128, :],
                                     ws[64:128, _WG + 64 * co:_WG + 64 * (co + 1)],
                                     z23, start=True, stop=True,
                                     tile_position=(64, 64))
                    q = qpool.tile([128, 2 * tc_], bf, tag="q")
                    if s < NSUP:
                        # one merged DVE op over both chunks: even chunk
                        # multiplies zz, odd chunk multiplies zzswap
                        q3 = q[:].rearrange("p (c t) -> p c t", c=2)
                        pt3 = pt[:].rearrange("p (c t) -> p c t", c=2)
                        zz3 = zzp[:].rearrange("p (c t) -> p c t", c=2)
                        nc.vector.tensor_mul(q3, pt3, zz3)
                    else:
                        nc.vector.tensor_mul(q[:, 0:tc_], pt[:, 0:tc_],
                                             zzp[:, 0:tc_])
                        nc.vector.tensor_mul(q[:, tc_:2 * tc_],
                                             pt[:, T:T + tc_],
                                             zzp[:, tc_:2 * tc_])
                    qs.append(q)
                return qs

            def emit_y(s, qs):
                tc_ = T if s < NSUP else TAIL
                psyf = pyp.tile([128, T], f32, tag="psy")
                psy = psyf[:, 0:tc_]
                for h in range(2):
                    for j in range(2):
                        c = 2 * h + j
                        nc.tensor.matmul(psy[:],
                                         ws[:, _WU + 128 * c:_WU + 128 * (c + 1)],
                                         qs[h][:, j * tc_:(j + 1) * tc_],
                                         start=(c == 0), stop=(c == 3))

                # stage output (bf16) and DMA in 4-supertile batches on the
                # sync HWDGE ring: FIFO order behind all queued inputs, so
                # outputs never steal HBM bandwidth from the input stream
                if s < NSUP:
                    if s % BATCH == 0:
                        yb = ybpool.tile([128, BATCH * T], bf, tag="yb")
                        yb_holder[0] = yb
                    yb = yb_holder[0]
                    nc.scalar.copy(yb[:, (s % BATCH) * T:(s % BATCH + 1) * T],
                                   psy[:])
                    if s % BATCH == BATCH - 1:
                        nc.sync.dma_start(ym[s // BATCH], yb[:])
                else:
                    ytb = ybpool.tile([128, TAIL], bf, tag="ytb")
                    nc.scalar.copy(ytb[:], psy[:])
                    nc.sync.dma_start(yt[:], ytb[:])

            # 2-deep software pipeline: PE queue per iteration is
            #   factors(s+1) | PT(s) | y(s-1)
            # so every matmul's cross-engine inputs are a full stage old.
            zz_prev, swap_prev = emit_front(order[0])
            swap_prev()
            q_prev = None
            for i in range(len(order)):
                if i + 1 < len(order):
                    zz_next, swap_next = emit_front(order[i + 1])
                else:
                    zz_next, swap_next = None, None
                q_cur = emit_pt(order[i], zz_prev)
                if q_prev is not None:
                    emit_y(order[i - 1], q_prev)
                if swap_next is not None:
                    swap_next()
                zz_prev = zz_next
                q_prev = q_cur
            emit_y(order[-1], q_prev)
    return nc


def _host_pack_weights(G, U_stack, U_output):
    U = np.asarray(U_stack, np.float32)
    Uo = np.asarray(U_output, np.float32)
    Gf = np.asarray(G, np.float32)
    wpk = np.zeros((128, _WCOLS), np.float32)
    W0 = np.repeat(U[0], 8, axis=1)            # [256,64] a -> U0[:, a//8]
    W1 = np.tile(U[1], (1, 8))                 # [256,64] a -> U1[:, a%8]
    W2 = np.repeat(U[2], 8, axis=1)            # [256,64] b -> U2[:, b//8]
    W3 = np.tile(U[3], (1, 8))                 # [256,64] b -> U3[:, b%8]
    for k in range(2):
        r = slice(128 * k, 128 * (k + 1))
        wpk[:, _WA + 128 * k:_WA + 128 * k + 64] = W0[r]
        wpk[:, _WA + 128 * k + 64:_WA + 128 * k + 128] = W2[r]
        wpk[:, _WB + 128 * k:_WB + 128 * k + 64] = W1[r]
        wpk[:, _WB + 128 * k + 64:_WB + 128 * k + 128] = W3[r]
    # Ghat[m1, a=(m0 m4), b=(m3 m2)] = G[m0,m1,m2,m3,m4]
    Ghat = np.ascontiguousarray(Gf.transpose(1, 0, 4, 3, 2)).reshape(8, 64, 64)
    for c in range(4):
        wpk[0:64, _WG + 64 * c:_WG + 64 * (c + 1)] = Ghat[c]
        wpk[64:128, _WG + 64 * c:_WG + 64 * (c + 1)] = Ghat[4 + c].T
        # even chunks use the [B | A] psum layout, odd chunks the flipped
        # [A | B] layout - the U_out rows swap accordingly
        lo, hi = (4 + c, c) if c % 2 == 0 else (c, 4 + c)
        wpk[0:64, _WU + 128 * c:_WU + 128 * (c + 1)] = Uo[lo][None, :]
        wpk[64:128, _WU + 128 * c:_WU + 128 * (c + 1)] = Uo[hi][None, :]
    return wpk.astype(BF16)


def _install_ntff_hook():
    import types
    if "antenv.axon_hooks" in sys.modules:
        return
    mod = types.ModuleType("antenv.axon_hooks")
    holder = {"hook": None}
    mod.set_axon_ntff_profile_hook = lambda h: holder.__setitem__("hook", h)
    mod.get_axon_ntff_profile_hook = lambda: holder["hook"]
    sys.modules["antenv.axon_hooks"] = mod
    import antenv
    antenv.axon_hooks = mod
    from trn_agent_boot.trn_boot import _ntff_profile_via_ctypes
    mod.set_axon_ntff_profile_hook(_ntff_profile_via_ctypes("/opt/axon/libaxon_pjrt.so"))


def _pack_inputs(X, wpb):
    in_maps = []
    for c in range(NCORES):
        sh = X[c * NPC:(c + 1) * NPC]                      # [6250, 4, 256]
        main = (sh[:NSUP * T]
                .reshape(NSUP, T, 4, 2, 128)
                .transpose(0, 4, 2, 3, 1)                  # [s, p, d, ch, t]
                .reshape(NSUP, 128, 8 * T))
        tail = (sh[NSUP * T:]
                .reshape(TAIL, 4, 2, 128)
                .transpose(3, 1, 2, 0)
                .reshape(128, 8 * TAIL))
        in_maps.append({
            "xm": np.ascontiguousarray(main).astype(BF16),
            "xt": np.ascontiguousarray(tail).astype(BF16),
            "wp": wpb,
        })
    return in_maps


_NC_CACHE = None


def kernel(neighbour_states, G, U_stack, U_output):
    global _NC_CACHE
    X = np.asarray(neighbour_states, np.float32)
    wpb = _host_pack_weights(G, U_stack, U_output)
    in_maps = _pack_inputs(X, wpb)

    if _NC_CACHE is None:
        _NC_CACHE = _build_nc()
    nc = _NC_CACHE

    trace = bool(os.environ.get("HOSVD_TRACE"))
    if trace:
        _install_ntff_hook()
    res = run_bass_kernel_spmd(nc, in_maps, core_ids=list(range(NCORES)),
                               trace=trace)
    if trace and res.exec_time_ns is not None:
        print(f"HW exec time: {res.exec_time_ns} ns")

    out = np.empty((N, 128), np.float32)
    for c in range(NCORES):
        ymc = np.asarray(res.results[c]["ym"]).astype(np.float32)  # [3,128,2048]
        ytc = np.asarray(res.results[c]["yt"]).astype(np.float32)  # [128,106]
        base = c * NPC
        out[base:base + NSUP * T] = ymc.transpose(0, 2, 1).reshape(NSUP * T, 128)
        out[base + NSUP * T:base + NPC] = ytc.T
    return out
